# revision 8
# baseline (speedup 1.0000x reference)
"""Trainium2 Bass kernel for nn_Attention_37855841747487.

Dense transformer attention block: QKV projection, per-head L2-norm with
gamma * sqrt(d), xPos rotary embedding, GQA softmax attention (16 q heads,
4 kv heads), output projection with residual + bias.

Sharding: 8 cores = 2 batches x 4 query-row slices of 512. Each core
computes K/V for its full batch (duplicated across the 4 cores of that
batch) and attention + output projection for its 512 query rows. No
collectives.

On-core layout: projections contract over dim=1024 (x^T built via bf16
DMA-transpose), norm+rope run in natural [token, dim] layout on DVE/ACT,
attention uses transposed scores S^T[keys, q] so softmax needs no
partition reductions: exp on ScalarE (bounded logits, no max pass) and
the softmax denominator comes from an extra ones-column appended to V.
All matmuls bf16 with fp32 PSUM accumulation (validated: rel l2 error
~1e-4 vs fp64 reference).
"""

import sys

sys.path.insert(0, "/opt/trn_rl_repo")

import numpy as np

B, N, DIM = 2, 2048, 1024
H, KVH, D = 16, 4, 64
XPOS_SB = 4096
QS = N // 4  # query rows per core
NCORES = 8

_CACHE = {}


# ---------------------------------------------------------------- host tables
def _make_tables(positions, scale_pow, gamma):
    """xPos rotary tables with rotate-half sign, gamma and rms folded in.

    Returns cosT, sinT of shape [n, Hg, 64]:
      roped(x) = l2norm(x) * cosT + swap_halves(l2norm(x)) * sinT
    where swap_halves swaps d<32 and d>=32.
    """
    d = D
    half = np.arange(0, d, 2, dtype=np.float64)
    inv_freq = 1.0 / (10000.0 ** (half / d))
    t = positions.astype(np.float64)
    freqs = t[:, None] * inv_freq[None, :]
    freqs = np.concatenate([freqs, freqs], axis=-1)
    base_scale = (half + 0.4 * d) / (1.4 * d)
    power = (t - N // 2) / XPOS_SB
    scale = base_scale[None, :] ** power[:, None]
    scale = np.concatenate([scale, scale], axis=-1)
    scale = scale**scale_pow
    cos = np.cos(freqs) * scale
    sin = np.sin(freqs) * scale
    sinA = np.concatenate([-sin[:, :32], sin[:, 32:]], axis=-1)
    rms = np.sqrt(np.float64(D))
    gswap = np.concatenate([gamma[:, 32:], gamma[:, :32]], axis=-1)
    cosT = cos[:, None, :] * (gamma[None, :, :] * rms)
    sinT = sinA[:, None, :] * (gswap[None, :, :] * rms)
    return cosT.astype(np.float32), sinT.astype(np.float32)


# ---------------------------------------------------------------- bass kernel
def _build_nc(ht, htk):
    """Trace + compile the per-core program. ht/htk: table head dims (1 when
    gamma is all-ones and the head axis broadcasts, else H / KVH)."""
    import concourse.bacc as bacc
    import concourse.bass as bass
    import concourse.mybir as mybir
    import concourse.tile as tile
    from concourse.masks import make_identity

    f32 = mybir.dt.float32
    bf16 = mybir.dt.bfloat16
    AF = mybir.ActivationFunctionType
    AX = mybir.AxisListType
    OP = mybir.AluOpType

    nc = bacc.Bacc("TRN2", target_bir_lowering=False, debug=False,
                   num_devices=NCORES)

    xb_d = nc.dram_tensor("xb", [N, DIM], f32, kind="ExternalInput")
    qx_d = nc.dram_tensor("qx", [QS, DIM], f32, kind="ExternalInput")
    wq_d = nc.dram_tensor("wq", [DIM, H * D], f32, kind="ExternalInput")
    wkv_d = nc.dram_tensor("wkv", [DIM, 2 * KVH * D], f32, kind="ExternalInput")
    wo_d = nc.dram_tensor("wo", [H * D, DIM], f32, kind="ExternalInput")
    bo_d = nc.dram_tensor("bo", [DIM], f32, kind="ExternalInput")
    tqc_d = nc.dram_tensor("tqc", [QS, ht, D], f32, kind="ExternalInput")
    tqs_d = nc.dram_tensor("tqs", [QS, ht, D], f32, kind="ExternalInput")
    tkc_d = nc.dram_tensor("tkc", [N, htk, D], f32, kind="ExternalInput")
    tks_d = nc.dram_tensor("tks", [N, htk, D], f32, kind="ExternalInput")
    y_d = nc.dram_tensor("y", [QS, DIM], f32, kind="ExternalOutput")

    from contextlib import ExitStack

    with tile.TileContext(nc) as tc, ExitStack() as ctx:
        persist = ctx.enter_context(tc.tile_pool(name="persist", bufs=1))
        dram = ctx.enter_context(tc.tile_pool(name="dram", bufs=1, space="DRAM"))

        # ---- persistent SBUF tensors
        wq_sb = persist.tile([128, 8, H * D], bf16)
        wkv_sb = persist.tile([128, 8, 2 * KVH * D], bf16)
        wo_sb = persist.tile([128, 8, DIM], bf16)
        xT_sb = persist.tile([128, 8, N], bf16)       # x_b^T
        qxT_sb = persist.tile([128, 8, QS], bf16)     # qx^T
        qT_sb = persist.tile([128, 8, QS], bf16)      # roped q^T
        kT_sb = persist.tile([128, 2, N], bf16)       # roped k^T
        v_sb = persist.tile([128, 16, KVH * (D + 1)], bf16)  # v natural + ones
        aoT_sb = persist.tile([128, 8, QS], bf16)     # attention out^T
        tqc_sb = persist.tile([128, 4, ht, D], f32)
        tqs_sb = persist.tile([128, 4, ht, D], f32)
        tkc_sb = persist.tile([128, 16, htk, D], f32)
        tks_sb = persist.tile([128, 16, htk, D], f32)
        bo_sb = persist.tile([128, 8], f32)
        ident = persist.tile([128, 128], f32)
        ones1 = persist.tile([1, D], f32)
        make_identity(nc, ident)
        nc.vector.memset(ones1, 1.0)
        v4 = v_sb.rearrange("p a (kv e) -> p a kv e", e=D + 1)
        nc.vector.memset(v4[:, :, :, D : D + 1], 1.0)

        # ---- weight / table / misc loads
        for kt in range(8):
            nc.gpsimd.dma_start(out=wq_sb[:, kt, :],
                                in_=wq_d[kt * 128 : (kt + 1) * 128, :])
            nc.gpsimd.dma_start(out=wkv_sb[:, kt, :],
                                in_=wkv_d[kt * 128 : (kt + 1) * 128, :])
            nc.gpsimd.dma_start(out=wo_sb[:, kt, :],
                                in_=wo_d[kt * 128 : (kt + 1) * 128, :])
        nc.sync.dma_start(out=tqc_sb, in_=tqc_d.rearrange("(a p) h d -> p a h d", p=128))
        nc.sync.dma_start(out=tqs_sb, in_=tqs_d.rearrange("(a p) h d -> p a h d", p=128))
        nc.sync.dma_start(out=tkc_sb, in_=tkc_d.rearrange("(a p) h d -> p a h d", p=128))
        nc.sync.dma_start(out=tks_sb, in_=tks_d.rearrange("(a p) h d -> p a h d", p=128))
        nc.sync.dma_start(out=bo_sb, in_=bo_d.rearrange("(m p) -> p m", p=128))

        # ---- A0: build x^T / qx^T via cast-DMA + bf16 DMA-transpose
        xbf = dram.tile([N, DIM], bf16)
        qxbf = dram.tile([QS, DIM], bf16)
        for i in range(4):
            nc.gpsimd.dma_start(out=xbf[i * 512 : (i + 1) * 512, :],
                                in_=xb_d[i * 512 : (i + 1) * 512, :])
        nc.gpsimd.dma_start(out=qxbf, in_=qx_d[:, :])
        for i in range(4):
            nc.sync.dma_start_transpose(out=xT_sb[:, :, i * 512 : (i + 1) * 512],
                                        in_=xbf[i * 512 : (i + 1) * 512, :])
        nc.sync.dma_start_transpose(out=qxT_sb, in_=qxbf[:, :])

        def norm_rope(pin, cos_t, sin_t, hout, A, Hh):
            """pin: PSUM fp32 [128, A, Hh, 64] projected tile (natural layout).
            cos_t/sin_t: [128, A, Hh, 64] table APs (may be broadcast views).
            hout: SBUF fp32 [128, A, Hh, 64] roped, normalized output."""
            sq = stage.tile([128, A, Hh, D], f32, tag="sq")
            nc.scalar.activation(sq, pin, AF.Square)
            ss = stage.tile([128, A, Hh], f32, tag="ss")
            nc.vector.tensor_reduce(ss, sq, axis=AX.X, op=OP.add)
            nrm = stage.tile([128, A, Hh], f32, tag="nrm")
            nc.scalar.activation(nrm, ss, AF.Sqrt)
            rs = stage.tile([128, A, Hh], f32, tag="rs")
            nc.vector.reciprocal(rs, nrm)
            rsb = rs.unsqueeze(3).broadcast_to([128, A, Hh, D])
            t1 = stage.tile([128, A, Hh, D], f32, tag="t1")
            nc.vector.tensor_tensor(out=t1, in0=pin, in1=rsb, op=OP.mult)
            # hout = t1 * cos + swap_halves(t1) * sin
            nc.vector.tensor_tensor(out=hout[:, :, :, 0:32],
                                    in0=t1[:, :, :, 32:64],
                                    in1=sin_t[:, :, :, 0:32], op=OP.mult)
            nc.vector.tensor_tensor(out=hout[:, :, :, 32:64],
                                    in0=t1[:, :, :, 0:32],
                                    in1=sin_t[:, :, :, 32:64], op=OP.mult)
            t2 = stage.tile([128, A, Hh, D], f32, tag="sq")
            nc.vector.tensor_tensor(out=t2, in0=t1, in1=cos_t, op=OP.mult)
            nc.vector.tensor_tensor(out=hout, in0=hout, in1=t2, op=OP.add)

        stage = ctx.enter_context(tc.tile_pool(name="stage", bufs=3))

        # ---- A1-Q: q projection + norm + rope + transpose to qT_sb
        with tc.tile_pool(name="q_ps", bufs=2, space="PSUM") as q_ps, \
             tc.tile_pool(name="qtp", bufs=2, space="PSUM") as qtp_ps:
            for m in range(4):
                qp = q_ps.tile([128, 2, 512], f32)
                for kt in range(8):
                    for nn in range(2):
                        nc.tensor.matmul(
                            qp[:, nn, :],
                            lhsT=qxT_sb[:, kt, m * 128 : (m + 1) * 128],
                            rhs=wq_sb[:, kt, nn * 512 : (nn + 1) * 512],
                            start=(kt == 0), stop=(kt == 7))
                qhat = stage.tile([128, 1, H, D], f32, tag="hat")
                qin = (qp.rearrange("p a b -> p (a b)")
                         .rearrange("p (o h d) -> p o h d", o=1, d=D))
                tqc_v = tqc_sb[:, m].unsqueeze(1)
                tqs_v = tqs_sb[:, m].unsqueeze(1)
                if ht == 1:
                    tqc_v = tqc_v.broadcast_to([128, 1, H, D])
                    tqs_v = tqs_v.broadcast_to([128, 1, H, D])
                norm_rope(qin, tqc_v, tqs_v, qhat, 1, H)
                qflat = qhat.rearrange("p o h d -> p (o h d)")
                for jg in range(2):
                    tp = qtp_ps.tile([128, 4, 128], f32)
                    for j4 in range(4):
                        j = jg * 4 + j4
                        nc.tensor.transpose(tp[:, j4, :],
                                            qflat[:, j * 128 : (j + 1) * 128],
                                            ident)
                    nc.scalar.copy(
                        out=qT_sb[:, jg * 4 : (jg + 1) * 4, m * 128 : (m + 1) * 128],
                        in_=tp)

        # ---- A1-KV: k/v projections; k: norm+rope+transpose; v: scatter+ones
        with tc.tile_pool(name="kv_ps", bufs=3, space="PSUM") as kv_ps, \
             tc.tile_pool(name="ktp", bufs=2, space="PSUM") as ktp_ps:
            for g in range(4):
                kp = kv_ps.tile([128, 4, KVH * D], f32, tag="kv")
                vp = kv_ps.tile([128, 4, KVH * D], f32, tag="kv")
                for i in range(4):
                    mt = g * 4 + i
                    for kt in range(8):
                        nc.tensor.matmul(
                            kp[:, i, :],
                            lhsT=xT_sb[:, kt, mt * 128 : (mt + 1) * 128],
                            rhs=wkv_sb[:, kt, 0 : KVH * D],
                            start=(kt == 0), stop=(kt == 7))
                        nc.tensor.matmul(
                            vp[:, i, :],
                            lhsT=xT_sb[:, kt, mt * 128 : (mt + 1) * 128],
                            rhs=wkv_sb[:, kt, KVH * D : 2 * KVH * D],
                            start=(kt == 0), stop=(kt == 7))
                # v evacuation into 65-column blocks (ones col pre-set)
                nc.scalar.copy(
                    out=v4[:, g * 4 : (g + 1) * 4, :, 0:D],
                    in_=vp.rearrange("p a (kv d) -> p a kv d", d=D))
                # k: norm + rope over the 4 m-tiles at once
                khat = stage.tile([128, 4, KVH, D], f32, tag="hat")
                kin = kp.rearrange("p a (h d) -> p a h d", d=D)
                tc_v = tkc_sb[:, g * 4 : (g + 1) * 4]
                ts_v = tks_sb[:, g * 4 : (g + 1) * 4]
                if htk == 1:
                    tc_v = tc_v.broadcast_to([128, 4, KVH, D])
                    ts_v = ts_v.broadcast_to([128, 4, KVH, D])
                norm_rope(kin, tc_v, ts_v, khat, 4, KVH)
                kflat = khat.rearrange("p a h d -> p a (h d)")
                for i in range(4):
                    mt = g * 4 + i
                    tp = ktp_ps.tile([128, 2, 128], f32)
                    for c in range(2):
                        nc.tensor.transpose(tp[:, c, :],
                                            kflat[:, i, c * 128 : (c + 1) * 128],
                                            ident)
                    nc.scalar.copy(out=kT_sb[:, :, mt * 128 : (mt + 1) * 128],
                                   in_=tp)

        # ---- B: attention per head
        groups = [(0, 3), (3, 6), (6, 9), (9, 12), (12, 15), (15, 16)]
        with tc.tile_pool(name="sT_ps", bufs=2, space="PSUM") as sT_ps, \
             tc.tile_pool(name="oT_ps", bufs=1, space="PSUM") as oT_ps, \
             tc.tile_pool(name="pT_pool", bufs=3) as pT_pool, \
             tc.tile_pool(name="small", bufs=3) as small:
            for h in range(H):
                kvh = h % KVH
                jq, qp_off = h // 2, 64 * (h % 2)
                ktile, kp_off = kvh // 2, 64 * (kvh % 2)
                oT = oT_ps.tile([D + 1, 512], f32)
                for (a, b) in groups:
                    ng = b - a
                    sT = sT_ps.tile([128, 3, 512], f32)
                    for i, kt in enumerate(range(a, b)):
                        nc.tensor.matmul(
                            sT[:, i, :],
                            lhsT=kT_sb[kp_off : kp_off + 64, ktile,
                                       kt * 128 : (kt + 1) * 128],
                            rhs=qT_sb[qp_off : qp_off + 64, jq, :],
                            start=True, stop=True)
                    pT = pT_pool.tile([128, 3, 512], bf16)
                    nc.scalar.activation(pT[:, 0:ng, :], sT[:, 0:ng, :],
                                         AF.Exp, scale=0.125)
                    for i, kt in enumerate(range(a, b)):
                        nc.tensor.matmul(
                            oT,
                            lhsT=v_sb[:, kt, kvh * (D + 1) : (kvh + 1) * (D + 1)],
                            rhs=pT[:, i, :],
                            start=(kt == 0), stop=(kt == 15))
                recip = small.tile([1, 512], f32, tag="recip")
                nc.vector.reciprocal(recip, oT[D : D + 1, :])
                rb = small.tile([D, 512], f32, tag="rb")
                nc.gpsimd.partition_broadcast(rb, recip)
                nc.vector.tensor_tensor(
                    out=aoT_sb[qp_off : qp_off + 64, jq, :],
                    in0=oT[0:D, :], in1=rb, op=OP.mult)

        # ---- C: output projection + bias + transpose + residual + store
        with tc.tile_pool(name="y_ps", bufs=2, space="PSUM") as y_ps, \
             tc.tile_pool(name="otp", bufs=2, space="PSUM") as otp_ps, \
             tc.tile_pool(name="cstage", bufs=1) as cstage, \
             tc.tile_pool(name="ystage", bufs=2) as ystage:
            y1_sb = cstage.tile([128, 8, QS], f32)     # y^T before final transpose
            qxf_sb = cstage.tile([128, 4, DIM], f32)   # qx natural (residual)
            nc.sync.dma_start(out=qxf_sb, in_=qx_d.rearrange("(a p) d -> p a d", p=128))
            for m in range(8):
                yp = y_ps.tile([128, 512], f32)
                for kt in range(8):
                    nc.tensor.matmul(yp,
                                     lhsT=wo_sb[:, kt, m * 128 : (m + 1) * 128],
                                     rhs=aoT_sb[:, kt, :],
                                     start=(kt == 0), stop=(kt == 7))
                nc.vector.tensor_scalar_add(y1_sb[:, m, :], in0=yp,
                                            scalar1=bo_sb[:, m : m + 1])
            for tq in range(4):
                ot = otp_ps.tile([128, 8, 128], f32)
                for m in range(8):
                    nc.tensor.transpose(ot[:, m, :],
                                        y1_sb[:, m, tq * 128 : (tq + 1) * 128],
                                        ident)
                yn = ystage.tile([128, DIM], f32)
                nc.vector.tensor_tensor(out=yn,
                                        in0=ot.rearrange("p a b -> p (a b)"),
                                        in1=qxf_sb[:, tq, :], op=OP.add)
                nc.sync.dma_start(out=y_d[tq * 128 : (tq + 1) * 128, :], in_=yn)

    nc.compile()
    return nc


def _get_nc(ht, htk):
    key = (ht, htk)
    if key not in _CACHE:
        _CACHE[key] = _build_nc(ht, htk)
    return _CACHE[key]


# ---------------------------------------------------------------- entry point
def make_in_maps(x, Wq, Wkv, q_gamma, k_gamma, Wo, bo):
    x = np.ascontiguousarray(np.asarray(x, dtype=np.float32))
    Wq = np.ascontiguousarray(np.asarray(Wq, dtype=np.float32))
    Wkv = np.ascontiguousarray(np.asarray(Wkv, dtype=np.float32))
    Wo = np.ascontiguousarray(np.asarray(Wo, dtype=np.float32))
    bo = np.ascontiguousarray(np.asarray(bo, dtype=np.float32))
    qg = np.asarray(q_gamma, dtype=np.float64).reshape(H, D)
    kg = np.asarray(k_gamma, dtype=np.float64).reshape(KVH, D)

    q_ones = np.allclose(qg, 1.0)
    k_ones = np.allclose(kg, 1.0)
    ht = 1 if q_ones else H
    htk = 1 if k_ones else KVH
    pos = np.arange(N)

    tkc, tks = _make_tables(pos, -1.0, kg[:1] * 0 + 1.0 if k_ones else kg)
    if k_ones:
        tkc, tks = tkc[:, :1], tks[:, :1]

    in_maps = []
    for c in range(NCORES):
        bi, qi = c // 4, c % 4
        qpos = pos[qi * QS : (qi + 1) * QS]
        tqc, tqs = _make_tables(qpos, +1.0, qg[:1] * 0 + 1.0 if q_ones else qg)
        if q_ones:
            tqc, tqs = tqc[:, :1], tqs[:, :1]
        in_maps.append({
            "xb": x[bi],
            "qx": np.ascontiguousarray(x[bi, qi * QS : (qi + 1) * QS]),
            "wq": Wq, "wkv": Wkv, "wo": Wo, "bo": bo,
            "tqc": np.ascontiguousarray(tqc), "tqs": np.ascontiguousarray(tqs),
            "tkc": np.ascontiguousarray(tkc), "tks": np.ascontiguousarray(tks),
        })
    return in_maps, (ht, htk)


def kernel(x, Wq, Wkv, q_gamma, k_gamma, Wo, bo):
    from concourse import bass_utils

    in_maps, (ht, htk) = make_in_maps(x, Wq, Wkv, q_gamma, k_gamma, Wo, bo)
    nc = _get_nc(ht, htk)
    res = bass_utils.run_bass_kernel_spmd(nc, in_maps,
                                          core_ids=list(range(NCORES)))
    out = np.zeros((B, N, DIM), np.float32)
    for c in range(NCORES):
        bi, qi = c // 4, c % 4
        out[bi, qi * QS : (qi + 1) * QS] = res.results[c]["y"]
    return out
